# revision 1
# baseline (speedup 1.0000x reference)
"""GNN message-passing encoder (PyG GraphConv x4) on 8 TRN2 NeuronCores.

Strategy (graph/data parallel, per sharding hint):
  - Nodes are sharded by destination range: core c owns dst nodes
    [c*6250, (c+1)*6250).  Each core processes the ~100K edges whose dst it
    owns.
  - Per layer the aggregation `segment_sum(x[src], dst)` runs as:
      gather rows of a replicated DRAM table via dma_gather (per-edge
      descriptors), then segment-sum on the PE with per-window one-hot
      indicator matmuls accumulated in PSUM (49 windows of 128 dst nodes).
    Indicators are built on the DVE with a broadcast is_equal against an
    iota tile.  Degree counts ride along as a ones-column of the L1 table.
  - Layer 1 aggregates x (64ch) directly; layer 2 aggregates h1 @ W2_rel
    (128ch, transform-before-aggregate); the two output heads share one
    aggregation of [h2@Wmu_rel | h2@Wls_rel] (32ch).
  - Between layers the per-core transformed tables are AllGathered so every
    core can gather any source row.  Weights are replicated.

dma_gather indices are int16, so each 50176-row table is split in two
halves with rebased indices; every (window, half) region is padded to a
fixed capacity so the single SPMD program fits all cores.
"""

import sys

sys.path.insert(0, "/opt/trn_rl_repo")

import numpy as np

import concourse.bass as bass
import concourse.bacc as bacc
import concourse.mybir as mybir
from concourse import tile
from concourse.bass import AP

# ---------------------------------------------------------------- constants
import os as _os

_SMALL = bool(int(_os.environ.get("GNN_SMALL", "0")))

IN_CH = 64
OC = 16
D1 = 256
D2 = 128
N_CORES = 8

if _SMALL:
    N_NODES = 4096
    N_EDGES = 16384
    LOCAL = 512
    NW = 4
    HALF_A = 2048
    CAP_A = 384
    CAP_B = 384
else:
    N_NODES = 50000
    N_EDGES = 800000
    LOCAL = N_NODES // N_CORES      # 6250
    NW = 49                         # dst windows of 128 per core
    HALF_A = 25088                  # balanced halves, both int16-addressable
    CAP_A = 1152                    # slots per (window, half-A); 9 chunks
    CAP_B = 1152                    # slots per (window, half-B); 9 chunks

LOCAL_PAD = NW * 128
RT = N_CORES * LOCAL_PAD
NCH_A = CAP_A // 128
NCH_B = CAP_B // 128

F32 = mybir.dt.float32
I16 = mybir.dt.int16

# weights blob layout: name -> (rows, cols)
_WB_SPEC = [
    ("iota", 128, 128),
    ("ident", 128, 128),
    ("Wstack1", 65, D1),    # [W1_rel; b1]
    ("W1_root", 64, D1),
    ("W2_rel_h0", 128, D2),
    ("W2_rel_h1", 128, D2),
    ("W2_root_h0", 128, D2),
    ("W2_root_h1", 128, D2),
    ("b2row", 1, D2),
    ("Wheads", D2, 2 * OC),  # [Wmu_rel | Wls_rel]
    ("Wroots", D2, 2 * OC),  # [Wmu_root | Wls_root]
    ("bheads", 1, 2 * OC),   # [bmu | bls]
]
_WB_OFF = {}
_off = 0
for _n, _r, _c in _WB_SPEC:
    _WB_OFF[_n] = (_off, _r, _c)
    _off += _r * _c
WB_LEN = _off

_NC_CACHE = None


def _bcast3(ap2d: AP, mid: int, inner: int, mode: str) -> AP:
    """[128, X] -> [128, mid, inner] with a 0-stride broadcast dim."""
    if mode == "inner0":   # [128, mid] -> [128, mid, inner], inner step 0
        return AP(ap2d.tensor, ap2d.offset, [ap2d.ap[0], ap2d.ap[1], [0, inner]])
    if mode == "mid0":     # [128, inner] -> [128, mid, inner], mid step 0
        return AP(ap2d.tensor, ap2d.offset, [ap2d.ap[0], [0, mid], ap2d.ap[1]])
    raise ValueError(mode)


def _build_nc():
    import os
    phases = int(os.environ.get("GNN_PHASES", "5"))
    nc = bacc.Bacc(None, num_swdge_queues=4, dynamic_dma_scratch_size=32768)

    # ---- IO
    x_pad = nc.dram_tensor("x_pad", [RT, 128], F32, kind="ExternalInput")
    x_tr_d = nc.dram_tensor("x_tr", [64, LOCAL_PAD], F32, kind="ExternalInput")
    gidxA = nc.dram_tensor("gidxA", [128, NW * CAP_A // 16], I16, kind="ExternalInput")
    gidxB = nc.dram_tensor("gidxB", [128, NW * CAP_B // 16], I16, kind="ExternalInput")
    drelA = nc.dram_tensor("drelA", [128, NW * NCH_A], F32, kind="ExternalInput")
    drelB = nc.dram_tensor("drelB", [128, NW * NCH_B], F32, kind="ExternalInput")
    wb = nc.dram_tensor("wb", [WB_LEN], F32, kind="ExternalInput")
    out_d = nc.dram_tensor("out", [LOCAL_PAD, 2 * OC], F32, kind="ExternalOutput")

    # ---- internal DRAM
    m2loc = nc.dram_tensor("m2loc", [LOCAL_PAD, D2], F32)
    tloc = nc.dram_tensor("tloc", [LOCAL_PAD, 64], F32)
    m2_full = nc.dram_tensor("m2_full", [RT, D2], F32, addr_space="Shared")
    t_full = nc.dram_tensor("t_full", [RT, 64], F32, addr_space="Shared")

    RG = [list(range(N_CORES))]

    with tile.TileContext(nc) as tc:
        with (
            tc.tile_pool(name="cst", bufs=1) as cst,
            tc.tile_pool(name="big", bufs=1) as big,
            tc.tile_pool(name="slab", bufs=2) as slabp,
            tc.tile_pool(name="ind", bufs=2) as indp,
            tc.tile_pool(name="sm", bufs=3) as sm,
            tc.tile_pool(name="stg", bufs=2) as stgp,
            tc.tile_pool(name="pw", bufs=3, space="PSUM") as pwp,
            tc.tile_pool(name="pt", bufs=1, space="PSUM") as ptp,
            tc.tile_pool(name="ph", bufs=2, space="PSUM") as php,
        ):
            # ---------------- constants / weights
            wt = {}
            for name, r, c in _WB_SPEC:
                t = cst.tile([r, c], F32, tag=f"w_{name}")
                off = _WB_OFF[name][0]
                nc.sync.dma_start(
                    t[:], wb[off:off + r * c].rearrange("(r c) -> r c", c=c)
                )
                wt[name] = t
            onesrow = cst.tile([1, 128], F32, tag="onesrow")
            nc.vector.memset(onesrow[:], 1.0)

            gixA = cst.tile([128, NW * CAP_A // 16], I16, tag="gixA")
            gixB = cst.tile([128, NW * CAP_B // 16], I16, tag="gixB")
            dstA = cst.tile([128, NW * NCH_A], F32, tag="dstA")
            dstB = cst.tile([128, NW * NCH_B], F32, tag="dstB")
            nc.sync.dma_start(gixA[:], gidxA[:])
            nc.sync.dma_start(gixB[:], gidxB[:])
            nc.sync.dma_start(dstA[:], drelA[:])
            nc.sync.dma_start(dstB[:], drelB[:])

            x_tr = big.tile([64, LOCAL_PAD], F32, tag="x_tr")
            nc.sync.dma_start(x_tr[:], x_tr_d[:])

            h1T0 = big.tile([128, LOCAL_PAD], F32, tag="h1T0")
            h1T1 = big.tile([128, LOCAL_PAD], F32, tag="h1T1")
            h2T = big.tile([128, LOCAL_PAD], F32, tag="h2T")
            dinv_all = big.tile([128, NW], F32, tag="dinv")

            iota_t = wt["iota"]
            ident = wt["ident"]

            # SWDGE queue discipline: Tile round-robins SWDGE completions over
            # 8 DMASW sem lanes in *scheduled* POOL order, and each lane must
            # stay on one queue.  So gathers are (a) chained with no-sync deps
            # to pin their POOL order to emission order and (b) issued on
            # queues rotating with period 4 (8 lanes % 4 queues -> each lane
            # sees exactly one queue).
            gchain = [None]
            gq = [0]

            def gather_chained(out_ap, table_ap, idx_ap, n, elem):
                ins = nc.gpsimd.dma_gather(
                    out_ap, table_ap, idx_ap, n, n, elem, queue_num=gq[0]
                )
                gq[0] = (gq[0] + 1) % 4
                if gchain[0] is not None:
                    bass._add_dep_helper(
                        ins.ins, gchain[0].ins, sync=False,
                        reason="pin SWDGE pool order for queue/lane pairing",
                    )
                gchain[0] = ins
                return ins

            def agg_window(w, tableA_ap, tableB_ap, elem, dcols, queue):
                """Gather + indicator matmuls for window w.
                Returns the accumulated PSUM tile [128, dcols].
                dma_gather is capped at 1024 indices per call (64 descriptors
                per DMA engine), so the A region is fetched in two calls."""
                slabA = slabp.tile([128, NCH_A, elem], F32, tag="slabA")
                a0 = w * (CAP_A // 16)
                for lo in range(0, CAP_A, 1024):
                    n = min(1024, CAP_A - lo)
                    gather_chained(
                        slabA[:, lo // 128:(lo + n) // 128, :], tableA_ap,
                        gixA[:, a0 + lo // 16:a0 + (lo + n) // 16], n, elem,
                    )
                slabB = slabp.tile([128, NCH_B, elem], F32, tag="slabB")
                for lo in range(0, CAP_B, 1024):
                    n = min(1024, CAP_B - lo)
                    gather_chained(
                        slabB[:, lo // 128:(lo + n) // 128, :], tableB_ap,
                        gixB[:, w * (CAP_B // 16) + lo // 16:
                             w * (CAP_B // 16) + (lo + n) // 16], n, elem,
                    )
                indA = indp.tile([128, NCH_A, 128], F32, tag="indA")
                nc.vector.tensor_tensor(
                    indA[:],
                    _bcast3(dstA[:, w * NCH_A:(w + 1) * NCH_A], NCH_A, 128, "inner0"),
                    _bcast3(iota_t[:], NCH_A, 128, "mid0"),
                    mybir.AluOpType.is_equal,
                )
                indB = indp.tile([128, NCH_B, 128], F32, tag="indB")
                nc.vector.tensor_tensor(
                    indB[:],
                    _bcast3(dstB[:, w * NCH_B:(w + 1) * NCH_B], NCH_B, 128, "inner0"),
                    _bcast3(iota_t[:], NCH_B, 128, "mid0"),
                    mybir.AluOpType.is_equal,
                )
                pw = pwp.tile([128, dcols], F32, tag="pw")
                nch = NCH_A + NCH_B
                k = 0
                for c in range(NCH_A):
                    nc.tensor.matmul(pw[:], indA[:, c, :], slabA[:, c, 0:dcols],
                                     start=(k == 0), stop=(k == nch - 1))
                    k += 1
                for c in range(NCH_B):
                    nc.tensor.matmul(pw[:], indB[:, c, :], slabB[:, c, 0:dcols],
                                     start=(k == 0), stop=(k == nch - 1))
                    k += 1
                return pw

            # ======================= L1 =======================
            for w in range(NW):
                ws = slice(w * 128, (w + 1) * 128)
                pw = agg_window(w, x_pad[0:HALF_A, :], x_pad[HALF_A:RT, :],
                                128, 65, w % 2)
                # deg -> deg_inv: recip(max(deg,1)) * min(deg,1)
                mx = sm.tile([128, 1], F32, tag="mx")
                nc.vector.tensor_scalar_max(mx[:], pw[:, 64:65], 1.0)
                rc = sm.tile([128, 1], F32, tag="rc")
                nc.vector.reciprocal(rc[:], mx[:])
                mn = sm.tile([128, 1], F32, tag="mn")
                nc.vector.tensor_scalar_min(mn[:], pw[:, 64:65], 1.0)
                dinv = sm.tile([128, 1], F32, tag="dinv_w")
                nc.vector.tensor_tensor(dinv[:], rc[:], mn[:], mybir.AluOpType.mult)
                nc.vector.tensor_copy(dinv_all[:, w:w + 1], dinv[:])
                # agg_nm = pw[:, :64] * dinv
                agg_nm = sm.tile([128, 64], F32, tag="agg_nm")
                nc.vector.tensor_scalar_mul(agg_nm[:], pw[:, 0:64], dinv[:])
                # aggT = [transpose(agg_nm); ones]
                ptr = ptp.tile([64, 128], F32, tag="ptr")
                nc.tensor.transpose(ptr[:], agg_nm[:], ident[:])
                aggT = sm.tile([65, 128], F32, tag="aggT")
                nc.vector.tensor_copy(aggT[0:64, :], ptr[:])
                nc.vector.memset(aggT[64:65, :], 1.0)
                # h1T halves
                for half, h1t in ((0, h1T0), (1, h1T1)):
                    hs = slice(half * 128, (half + 1) * 128)
                    ph = php.tile([128, 128], F32, tag="ph")
                    nc.tensor.matmul(ph[:], wt["Wstack1"][:, hs], aggT[:],
                                     start=True, stop=False)
                    nc.tensor.matmul(ph[:], wt["W1_root"][:, hs], x_tr[:, ws],
                                     start=False, stop=True)
                    nc.scalar.activation(
                        h1t[:, ws], ph[:], mybir.ActivationFunctionType.Relu
                    )
                # m2 for this window rides inside the L1 loop so its PE and
                # DMA work overlaps the gather stream
                if phases >= 2:
                    pm = php.tile([128, D2], F32, tag="ph")
                    nc.tensor.matmul(pm[:], h1T0[:, ws], wt["W2_rel_h0"][:],
                                     start=True, stop=False)
                    nc.tensor.matmul(pm[:], h1T1[:, ws], wt["W2_rel_h1"][:],
                                     start=False, stop=True)
                    stg = stgp.tile([128, D2], F32, tag="m2stg")
                    nc.vector.tensor_copy(stg[:], pm[:])
                    nc.sync.dma_start(m2loc[w * 128:(w + 1) * 128, :], stg[:])

            if phases >= 2:
                nc.gpsimd.collective_compute(
                    "AllGather", mybir.AluOpType.bypass, replica_groups=RG,
                    ins=[m2loc[:]], outs=[m2_full[:]],
                )
            # ======================= L2 + h2 =======================
            for w in range(NW if phases >= 3 else 0):
                ws = slice(w * 128, (w + 1) * 128)
                pw = agg_window(w, m2_full[0:HALF_A, :], m2_full[HALF_A:RT, :],
                                D2, D2, w % 2)
                agg2 = sm.tile([128, D2], F32, tag="agg2")
                nc.vector.tensor_scalar_mul(agg2[:], pw[:], dinv_all[:, w:w + 1])
                # transpose must be its own PSUM group (mixing transpose-mode
                # into an accumulation group poisons PSUM on HW)
                ptr2 = ptp.tile([128, 128], F32, tag="ptr2")
                nc.tensor.transpose(ptr2[:], agg2[:], ident[:])
                ph2 = php.tile([128, 128], F32, tag="ph")
                nc.tensor.matmul(ph2[:], wt["W2_root_h0"][:], h1T0[:, ws],
                                 start=True, stop=False)
                nc.tensor.matmul(ph2[:], wt["W2_root_h1"][:], h1T1[:, ws],
                                 start=False, stop=False)
                nc.tensor.matmul(ph2[:], wt["b2row"][:], onesrow[:],
                                 start=False, stop=True)
                tr2 = sm.tile([128, 128], F32, tag="tr2")
                nc.vector.tensor_copy(tr2[:], ptr2[:])
                hsum = sm.tile([128, 128], F32, tag="hsum")
                nc.vector.tensor_tensor(hsum[:], tr2[:], ph2[:],
                                        mybir.AluOpType.add)
                nc.scalar.activation(
                    h2T[:, ws], hsum[:], mybir.ActivationFunctionType.Relu
                )
                if phases >= 4:
                    pt = php.tile([128, 2 * OC], F32, tag="ph")
                    nc.tensor.matmul(pt[:], h2T[:, ws], wt["Wheads"][:],
                                     start=True, stop=True)
                    stg = stgp.tile([128, 64], F32, tag="tstg")
                    nc.vector.tensor_copy(stg[:, 0:2 * OC], pt[:])
                    nc.vector.memset(stg[:, 2 * OC:64], 0.0)
                    nc.sync.dma_start(tloc[w * 128:(w + 1) * 128, :], stg[:])
            if phases >= 4:
                nc.gpsimd.collective_compute(
                    "AllGather", mybir.AluOpType.bypass, replica_groups=RG,
                    ins=[tloc[:]], outs=[t_full[:]],
                )

            # ======================= heads =======================
            for w in range(NW if phases >= 5 else 0):
                ws = slice(w * 128, (w + 1) * 128)
                pw = agg_window(w, t_full[0:HALF_A, :], t_full[HALF_A:RT, :],
                                64, 2 * OC, w % 2)
                pf = php.tile([128, 2 * OC], F32, tag="ph")
                nc.tensor.matmul(pf[:], h2T[:, ws], wt["Wroots"][:],
                                 start=True, stop=False)
                nc.tensor.matmul(pf[:], onesrow[:], wt["bheads"][:],
                                 start=False, stop=True)
                aggh = sm.tile([128, 2 * OC], F32, tag="aggh")
                nc.vector.tensor_scalar_mul(aggh[:], pw[:], dinv_all[:, w:w + 1])
                ot = stgp.tile([128, 2 * OC], F32, tag="ot")
                nc.vector.tensor_tensor(ot[:], aggh[:], pf[:], mybir.AluOpType.add)
                nc.sync.dma_start(out_d[w * 128:(w + 1) * 128, :], ot[:])
            if phases < 5:
                dbg_src = {1: h1T0, 2: h1T0, 3: h2T, 4: h2T}[phases]
                for w in range(NW):
                    nc.sync.dma_start(
                        out_d[w * 128:(w + 1) * 128, :],
                        dbg_src[0:128, w * 128:w * 128 + 2 * OC].rearrange(
                            "p d -> d p"
                        ) if False else dbg_src[:, w * 128:(w + 1) * 128][:, 0:2 * OC],
                    )

    nc.compile()
    return nc


def get_nc():
    global _NC_CACHE
    if _NC_CACHE is None:
        _NC_CACHE = _build_nc()
    return _NC_CACHE


# ---------------------------------------------------------------- host prep

def _wrap_idx16(vals: np.ndarray, nslots: int) -> np.ndarray:
    """Slot-ordered int16 values -> [128, nslots/16] wrapped+tiled layout."""
    a = vals.astype(np.int16).reshape(nslots // 16, 16).T  # [16, W]
    return np.tile(a, (8, 1))


def _prep_core(src_row, dst_local, core_mask):
    """Build gidxA/B, drelA/B arrays for one core."""
    row = src_row[core_mask]
    dl = dst_local[core_mask]
    win = dl >> 7
    rel = (dl & 127).astype(np.float32)

    out = {}
    for half, cap, nch in ((0, CAP_A, NCH_A), (1, CAP_B, NCH_B)):
        sel = (row < HALF_A) if half == 0 else (row >= HALF_A)
        r = row[sel] - (0 if half == 0 else HALF_A)
        wv = win[sel]
        rv = rel[sel]
        order = np.argsort(wv, kind="stable")
        r, wv, rv = r[order], wv[order], rv[order]
        counts = np.bincount(wv, minlength=NW)
        if counts.max() > cap:
            raise RuntimeError(f"window overflow: {counts.max()} > {cap}")
        starts = np.zeros(NW, np.int64)
        starts[1:] = np.cumsum(counts)[:-1]
        pos = np.arange(len(wv)) - np.repeat(starts, counts)
        slot = wv * cap + pos
        nslots = NW * cap
        gvals = np.zeros(nslots, np.int64)
        gvals[slot] = r
        dvals = np.full(nslots, -1.0, np.float32)
        dvals[slot] = rv
        gname = "gidxA" if half == 0 else "gidxB"
        dname = "drelA" if half == 0 else "drelB"
        out[gname] = _wrap_idx16(gvals, nslots)
        out[dname] = dvals.reshape(nslots // 128, 128).T.copy()
    return out


def _pack_weights(i):
    wb = np.zeros(WB_LEN, np.float32)

    def put(name, arr):
        off, r, c = _WB_OFF[name]
        wb[off:off + r * c] = np.asarray(arr, np.float32).reshape(r * c)

    put("iota", np.tile(np.arange(128, dtype=np.float32), (128, 1)))
    put("ident", np.eye(128, dtype=np.float32))
    put("Wstack1", np.concatenate([i["W1_rel"], i["b1"][None, :]], 0))
    put("W1_root", i["W1_root"])
    put("W2_rel_h0", i["W2_rel"][0:128])
    put("W2_rel_h1", i["W2_rel"][128:256])
    put("W2_root_h0", i["W2_root"][0:128])
    put("W2_root_h1", i["W2_root"][128:256])
    put("b2row", i["b2"][None, :])
    put("Wheads", np.concatenate([i["Wmu_rel"], i["Wls_rel"]], 1))
    put("Wroots", np.concatenate([i["Wmu_root"], i["Wls_root"]], 1))
    put("bheads", np.concatenate([i["bmu"], i["bls"]])[None, :])
    return wb


def kernel(**inputs):
    x = np.asarray(inputs["x"], np.float32)
    ei = np.asarray(inputs["edge_index"])
    src = ei[0].astype(np.int64)
    dst = ei[1].astype(np.int64)

    owner = dst // LOCAL
    dst_local = dst - owner * LOCAL
    src_row = (src // LOCAL) * LOCAL_PAD + (src % LOCAL)

    x_pad = np.zeros((RT, 128), np.float32)
    for c in range(N_CORES):
        x_pad[c * LOCAL_PAD:c * LOCAL_PAD + LOCAL, 0:64] = x[c * LOCAL:(c + 1) * LOCAL]
    x_pad[:, 64] = 1.0

    wb = _pack_weights({k: np.asarray(v, np.float32) for k, v in inputs.items()
                        if k not in ("x", "edge_index")})

    in_maps = []
    for c in range(N_CORES):
        m = _prep_core(src_row, dst_local, owner == c)
        x_tr = np.zeros((64, LOCAL_PAD), np.float32)
        x_tr[:, :LOCAL] = x[c * LOCAL:(c + 1) * LOCAL].T
        m["x_pad"] = x_pad
        m["x_tr"] = x_tr
        m["wb"] = wb
        in_maps.append(m)

    from concourse.bass_utils import run_bass_kernel_spmd

    nc = get_nc()
    res = run_bass_kernel_spmd(nc, in_maps, list(range(N_CORES)))

    mu = np.zeros((N_NODES, OC), np.float32)
    ls = np.zeros((N_NODES, OC), np.float32)
    for c in range(N_CORES):
        o = res.results[c]["out"][:LOCAL]
        mu[c * LOCAL:(c + 1) * LOCAL] = o[:, :OC]
        ls[c * LOCAL:(c + 1) * LOCAL] = o[:, OC:]
    return (mu, ls)


if __name__ == "__main__":
    # quick self-test with random data
    rng = np.random.default_rng(0)
    ins = {
        "x": rng.standard_normal((N_NODES, IN_CH)).astype(np.float32),
        "edge_index": rng.integers(0, N_NODES, (2, N_EDGES)),
        "W1_rel": rng.standard_normal((IN_CH, D1)).astype(np.float32) * 0.1,
        "b1": np.zeros(D1, np.float32),
        "W1_root": rng.standard_normal((IN_CH, D1)).astype(np.float32) * 0.1,
        "W2_rel": rng.standard_normal((D1, D2)).astype(np.float32) * 0.1,
        "b2": np.zeros(D2, np.float32),
        "W2_root": rng.standard_normal((D1, D2)).astype(np.float32) * 0.1,
        "Wmu_rel": rng.standard_normal((D2, OC)).astype(np.float32) * 0.1,
        "bmu": np.zeros(OC, np.float32),
        "Wmu_root": rng.standard_normal((D2, OC)).astype(np.float32) * 0.1,
        "Wls_rel": rng.standard_normal((D2, OC)).astype(np.float32) * 0.1,
        "bls": np.zeros(OC, np.float32),
        "Wls_root": rng.standard_normal((D2, OC)).astype(np.float32) * 0.1,
    }
    mu, ls = kernel(**ins)
    print("kernel ran:", mu.shape, ls.shape, mu[:2, :4])

